# revision 2
# baseline (speedup 1.0000x reference)
"""AttnBlock (GroupNorm + single-head self-attention + residual) on 8 trn2 cores.

Sharding: data-parallel over batch (4 samples) x query-halves (2) = 8 cores.
Each core gets its sample's tokens rotated so its 2048 queries are tokens
0:2048 of its input; GroupNorm stats and attention key-sums are invariant to
token order.

Weight reassociation (host precomputes weight-only products): scores
S = hn (Wq Wk^T) hn^T = x M2 x^T + per-key bias (M2 = diag(a) M diag(a)
absorbs the GroupNorm affine; per-query terms cancel in softmax); values
attn hn Wv Wo = (attn x) diag(a)(Wv Wo) + const row.

fp8 DoubleRow everywhere: x ships as E4M3 (x^T for scores/projections,
token-major for attnV values); M2/W2~ are built on device as E4M3 with a
2^5 prescale that cancels between the PT8 copy (a/32) and the osb8 drain
(2^-5).  All heavy matmuls contract 256 channels/instruction.  exp runs on
ACT, writing unnormalized weights at = 2 exp(SCALE*S + v - 3) straight to
E4M3 (max |at| ~ 45 << 240); sum-exp accumulates on the PE via a
zero-padded [128,2,128] ones lhsT DoubleRow matmul (dual-fp8 Ldweights
requires full column blocks) whose row 0 is the sum.  Softmax normalization
is deferred to after the output projection (queries are then the partition
dim).

GroupNorm stats: per-chunk moments (DVE bn_stats + ACT raw-sum tail), then
ONE batched group-reduce chain over all 4 chunks ([8,CH]/[128,CH] tiles)
instead of a per-chunk scalar chain.  The per-key exp bias v is computed
flipped (v^T = wv2^T x~^T, 16 DoubleRow matmuls at N=512) and transposed to
per-partition layout via a DRAM bounce, replacing 64 LDW-bound tiny matmuls.

Per 512-query block: 16 key-chunk pairs x (4 score + 4 attnV + 1 sumexp)
DoubleRow matmuls; the output stage of block b is emitted interleaved into
block b+1's pair loop.  PSUM: 4 ot + 1 se fixed, and a 3-deep rotation
shared by score tiles, rse transposes and the fp output matmuls.
"""
import os
import math
import numpy as np
import ml_dtypes
from contextlib import ExitStack

import concourse.bass as bass
import concourse.tile as tile
from concourse import bacc, mybir
from concourse.bass_utils import run_bass_kernel_spmd

B, H, W, C = 4, 64, 64, 512
N = H * W            # 4096 tokens / sample
NQ = N // 2          # 2048 queries / core
G = 32
GS = C // G          # 16 channels / group
EPS = 1e-6
CH = C // 128        # 4 channel chunks
CP = CH // 2         # 2 channel chunk-pairs
KC = N // 128        # 32 key chunks
KP = KC // 2         # 16 key chunk-pairs
QB = NQ // 512       # 4 query blocks
SCALE = float(C) ** -0.5

C0 = 3.0             # at = 2 exp(SCALE*S + v - 3): range ~[0, 46] in E4M3
ATB = math.log(2.0) - C0
WV2S = 1024.0        # wv2 prescale (entries ~5e-4, below E4M3 subnormals)
LOG2E = 1.0 / math.log(2.0)
SCH_DELTA = 0.25     # trunc-rounding calibration for the Schraudolph pairs
# key-chunk pairs whose exp runs as a Schraudolph e5m2 bit-trick off ACT:
SCH_PAIRS = {4: "dve", 9: "dve", 14: "dve"} \
    if os.environ.get("SCHRAUD", "1") == "1" else {}

F32 = mybir.dt.float32
BF = mybir.dt.float16
E4 = mybir.dt.float8e4
E5 = mybir.dt.float8e5
U8 = mybir.dt.uint8
A = mybir.AluOpType
AF = mybir.ActivationFunctionType
DRM = mybir.MatmulPerfMode.DoubleRow

_CACHE = {}


def _build():
    nc = bacc.Bacc("TRN2", target_bir_lowering=False, debug=False, num_devices=8)
    xt8_d = nc.dram_tensor("xt8", [128, CH, N], E4, kind="ExternalInput").ap()
    xk8_d = nc.dram_tensor("xk8", [128, KC, C], E4, kind="ExternalInput").ap()
    xres = nc.dram_tensor("xres", [NQ, C], F32, kind="ExternalInput").ap()
    wm_d = nc.dram_tensor("wm", [128, 2, CH, C], BF, kind="ExternalInput").ap()
    pc_d = nc.dram_tensor("pc", [128, 3 * CH + 8], F32, kind="ExternalInput").ap()
    gmat2_d = nc.dram_tensor("gmat2", [8, 128], F32, kind="ExternalInput").ap()
    rows_d = nc.dram_tensor("rows", [1, C], F32, kind="ExternalInput").ap()
    out = nc.dram_tensor("out", [NQ, C], F32, kind="ExternalOutput").ap()

    with tile.TileContext(nc) as tc, ExitStack() as ctx:
        pers = ctx.enter_context(tc.tile_pool(name="pers", bufs=1))
        dram = ctx.enter_context(tc.tile_pool(name="dram", bufs=2, space="DRAM"))

        # ---------- phase 1: x^T fp8 first (gates stats + all matmuls) ------
        xT = pers.tile([128, CH, N], E4, name="xT")
        engs = [nc.sync, nc.scalar, nc.gpsimd, nc.sync]
        for j in range(CH):
            for h in range(4):
                sl = slice(h * N // 4, (h + 1) * N // 4)
                engs[h].dma_start(out=xT[:, j, sl], in_=xt8_d[:, j, sl])

        pc_sb = pers.tile([128, 3 * CH + 8], F32, name="pc_sb")
        nc.sync.dma_start(out=pc_sb, in_=pc_d)
        gns_pc = pc_sb[:, 0:CH]
        gnb_pc = pc_sb[:, CH:2 * CH]
        g_pc = pc_sb[:, 2 * CH:3 * CH]
        gmat_sb = pc_sb[:, 3 * CH:3 * CH + 8]
        gmat2_sb = pers.tile([8, 128], F32, name="gmat2_sb")
        nc.sync.dma_start(out=gmat2_sb, in_=gmat2_d)
        rows_sb = pers.tile([1, C], F32, name="rows_sb")
        nc.sync.dma_start(out=rows_sb, in_=rows_d)
        h2_row = rows_sb[:, 0:C]
        wm_sb = pers.tile([128, 2, CH, C], BF, name="wm_sb")
        for k in range(2):
            nc.gpsimd.dma_start(out=wm_sb[:, k], in_=wm_d[:, k])
        ident1 = pers.tile([1, 1], F32, name="ident1")
        nc.vector.memset(ident1, 1.0)
        # dual-fp8 Ldweights requires full column blocks: pad the sum-exp
        # ones vector to [128, 2, 128] with only column 0 nonzero
        ones8 = pers.tile([128, 2, 128], E4, name="ones8")
        nc.vector.memset(ones8, 0.0)
        nc.vector.memset(ones8[:, :, 0:1], 1.0)
        xtok = pers.tile([128, KC, C], E4, name="xtok")

        m8 = pers.tile([128, CH, C], E4, name="m8")      # 32 * a() M
        w28 = pers.tile([128, CH, C], E4, name="w28")    # 32 * a() (Wv Wo)
        a_pc = pers.tile([128, CH], F32, name="a_pc")
        ad32 = pers.tile([128, CH], F32, name="ad32")    # a/32 (PT8 copy scale)
        d_pc = pers.tile([128, CH], F32, name="d_pc")
        v_pc = pers.tile([128, KC], F32, name="v_pc")
        w5_pc = pers.tile([128, KC], F32, name="w5_pc")

        with tc.tile_pool(name="stats", bufs=2) as stp, \
             tc.tile_pool(name="stps", bufs=1, space="PSUM") as stps:
            eps_t = stp.tile([8, 1], F32, tag="eps", bufs=1)
            nc.vector.memset(eps_t, float(GS * GS) * EPS)
            NS_DVE = 5                      # slices 0..4 on DVE bn_stats
            NA = NS_DVE * 512               # tokens on the DVE side
            NB = N - NA                     # tokens on the ACT accum side
            m2_all = stp.tile([128, CH, 2], F32, tag="m2a", bufs=1)
            for j in range(CH):
                st = stp.tile([128, NS_DVE, 6], F32, tag="st")
                xv = xT[:, j, :].rearrange("p (s f) -> p s f", f=512)
                for s in range(NS_DVE):
                    nc.vector.bn_stats(out=st[:, s, :], in_=xv[:, s, :])
                # raw moments of the tail tokens on the (otherwise idle) ACT
                junk = stp.tile([128, NB], BF, tag="junk")
                s1b = stp.tile([128, 2], F32, tag="s1b")
                nc.scalar.activation(out=junk, in_=xT[:, j, NA:N], func=AF.Copy,
                                     accum_out=s1b[:, 0:1])
                nc.scalar.activation(out=junk, in_=xT[:, j, NA:N], func=AF.Square,
                                     accum_out=s1b[:, 1:2])
                mv = stp.tile([128, 2], F32, tag="mv")
                nc.vector.bn_aggr(out=mv, in_=st)
                # m2 = [mean, E[x^2]] over all N tokens, merging both halves
                m2 = m2_all[:, j, :]
                nc.vector.tensor_mul(out=m2[:, 1:2], in0=mv[:, 0:1], in1=mv[:, 0:1])
                nc.vector.tensor_add(out=m2[:, 1:2], in0=m2[:, 1:2], in1=mv[:, 1:2])
                nc.vector.tensor_scalar(out=m2[:, 1:2], in0=m2[:, 1:2],
                                        scalar1=float(NA) / N, scalar2=None,
                                        op0=A.mult)
                nc.vector.tensor_scalar(out=m2[:, 0:1], in0=mv[:, 0:1],
                                        scalar1=float(NA) / N, scalar2=None, op0=A.mult)
                sb2 = stp.tile([128, 2], F32, tag="sb2")
                nc.vector.tensor_scalar(out=sb2, in0=s1b, scalar1=1.0 / N,
                                        scalar2=None, op0=A.mult)
                nc.vector.tensor_add(out=m2, in0=m2, in1=sb2)
            # batched group reduce: [8, CH, 2] group sums for all chunks
            #   rstd path: t3 = GS*S2 - S1^2 = GS^2 var; r = rstd/GS
            gsum = stps.tile([8, CH, 2], F32, tag="gsum", bufs=1)
            nc.tensor.matmul(out=gsum, lhsT=gmat_sb,
                             rhs=m2_all.rearrange("p a b -> p (a b)"),
                             start=True, stop=True)
            gg = stp.tile([8, CH, 2], F32, tag="gg", bufs=1)
            nc.vector.tensor_copy(out=gg, in_=gsum)
            t1 = stp.tile([8, CH], F32, tag="t1", bufs=1)
            nc.vector.tensor_mul(out=t1, in0=gg[:, :, 0], in1=gg[:, :, 0])
            t3 = stp.tile([8, CH], F32, tag="t3", bufs=1)
            nc.vector.tensor_scalar(out=t3, in0=gg[:, :, 1], scalar1=float(GS),
                                    scalar2=None, op0=A.mult)
            nc.vector.tensor_sub(out=t3, in0=t3, in1=t1)
            sq = stp.tile([8, CH], F32, tag="sq", bufs=1)
            nc.scalar.activation(out=sq, in_=t3, func=AF.Sqrt, bias=eps_t)
            nc.vector.reciprocal(out=gg[:, :, 1], in_=sq)   # r = rstd/16
            bc = stps.tile([128, CH, 2], F32, tag="bc", bufs=1)
            nc.tensor.matmul(out=bc, lhsT=gmat2_sb,
                             rhs=gg.rearrange("p a b -> p (a b)"),
                             start=True, stop=True)
            # bc0 = S1_pc = 16*mean_c ; bc1 = r_pc = rstd_c/16
            nc.vector.tensor_mul(out=a_pc, in0=bc[:, :, 1], in1=gns_pc)
            nc.vector.tensor_scalar(out=a_pc, in0=a_pc, scalar1=float(GS),
                                    scalar2=None, op0=A.mult)
            nc.vector.tensor_mul(out=d_pc, in0=bc[:, :, 0], in1=a_pc)
            nc.vector.tensor_scalar(out=d_pc, in0=d_pc,
                                    scalar1=-1.0 / GS, scalar2=None, op0=A.mult)
            nc.vector.tensor_add(out=d_pc, in0=d_pc, in1=gnb_pc)
            nc.vector.tensor_scalar(out=ad32, in0=a_pc,
                                    scalar1=1.0 / 32.0, scalar2=None, op0=A.mult)
            a32_pc = stp.tile([128, CH], F32, tag="a32", bufs=1)
            nc.vector.tensor_scalar(out=a32_pc, in0=a_pc, scalar1=32.0,
                                    scalar2=None, op0=A.mult)
            for j in range(CH):
                nc.vector.tensor_scalar(out=m8[:, j, :], in0=wm_sb[:, 0, j, :],
                                        scalar1=a32_pc[:, j:j + 1], scalar2=None,
                                        op0=A.mult)
                nc.scalar.activation(out=w28[:, j, :], in_=wm_sb[:, 1, j, :],
                                     func=AF.Copy, scale=a32_pc[:, j:j + 1])
            # d/a (output-side bias lhsT), raw d (v-vector lhsT), both fp16
            ra_pc = stp.tile([128, CH], F32, tag="ra_pc", bufs=1)
            nc.vector.reciprocal(out=ra_pc, in_=a_pc)
            nc.vector.tensor_mul(out=ra_pc, in0=ra_pc, in1=d_pc)
            d_bf = stp.tile([128, CH], BF, tag="d_bf", bufs=1)
            nc.vector.tensor_copy(out=d_bf, in_=ra_pc)            # d/a
            draw_bf = stp.tile([128, CH], BF, tag="draw_bf", bufs=1)
            nc.vector.tensor_copy(out=draw_bf, in_=d_pc)          # raw d
            # (M^T d) in per-partition layout via tiny column matmuls
            mtd_ps = stps.tile([128, CH], F32, tag="mtd_ps")
            for jc2 in range(CH):
                for jc in range(CH):
                    nc.tensor.matmul(out=mtd_ps[:, jc2:jc2 + 1],
                                     lhsT=wm_sb[:, 0, jc, 128 * jc2:128 * (jc2 + 1)],
                                     rhs=draw_bf[:, jc:jc + 1],
                                     start=(jc == 0), stop=(jc == CH - 1))
            # wv2 = WV2S * SCALE * a (.) (M^T d + g): lhsT column (16B step)
            # for the flipped v^T matmuls
            wv2_pc = stp.tile([128, CH], F32, tag="wv2_pc", bufs=1)
            nc.vector.tensor_add(out=wv2_pc, in0=mtd_ps, in1=g_pc)
            nc.vector.tensor_mul(out=wv2_pc, in0=wv2_pc, in1=a_pc)
            nc.vector.tensor_scalar(out=wv2_pc, in0=wv2_pc, scalar1=SCALE * WV2S,
                                    scalar2=None, op0=A.mult)
            wv2_16 = pers.tile([128, CH, 16], E4, name="wv2_16")
            nc.vector.tensor_copy(
                out=wv2_16[:, :, 0:1],
                in_=wv2_pc.rearrange("p (t o) -> p t o", o=1))
            # b2 = (d/a) @ W2~ + h2  (per-output-channel bias incl. bo, bv@Wo)
            b2_ps = stps.tile([1, C], F32, tag="b2_ps", name="b2_ps")
            for j in range(CH):
                nc.tensor.matmul(out=b2_ps, lhsT=d_bf[:, j:j + 1], rhs=wm_sb[:, 1, j, :],
                                 start=(j == 0), stop=(j == CH - 1))
            b2_row = stp.tile([1, C], F32, tag="b2_row", bufs=1)
            nc.vector.tensor_add(out=b2_row, in0=b2_ps, in1=h2_row)
            scr_b = dram.tile([1, C], F32, name="scr_b", bufs=1)
            nc.sync.dma_start(out=scr_b, in_=b2_row)
            b2_bc = pers.tile([128, C], F32, name="b2_bc")
            src_b = bass.AP(tensor=scr_b.tensor, offset=scr_b.offset,
                            ap=[[0, 128], [1, C]])
            nc.gpsimd.dma_start(out=b2_bc, in_=src_b)

        # token-major raw x fp8 (attnV values; needed only from attention)
        for q in range(4):
            eng = nc.scalar if q % 2 == 0 else nc.gpsimd
            eng.dma_start(out=xtok[:, 8 * q:8 * (q + 1), :],
                          in_=xk8_d[:, 8 * q:8 * (q + 1), :])

        # ---------- phase 2: flipped v^T matvec, then P projection ----------
        PT = [pers.tile([128, CH, 512], E4, name=f"PT{t}") for t in range(QB)]
        with tc.tile_pool(name="pps", bufs=6, space="PSUM") as pps:
            # v^T = wv2^T x~^T over all keys: [1, N] in 512-chunks, then a
            # DRAM bounce to the per-partition [128, KC] exp-bias layout
            vT_sb = pers.tile([1, N], F32, name="vT_sb")
            for s in range(KC // 4):
                vps = pps.tile([1, 512], F32, tag="vps", bufs=2, name=f"vps{s}")
                for jp in range(CP):
                    nc.tensor.matmul(out=vps,
                                     lhsT=wv2_16[:, 2 * jp:2 * jp + 2, 0:1],
                                     rhs=xT[:, 2 * jp:2 * jp + 2, 512 * s:512 * (s + 1)],
                                     start=(jp == 0), stop=(jp == CP - 1), perf_mode=DRM)
                nc.vector.tensor_copy(out=vT_sb[:, 512 * s:512 * (s + 1)], in_=vps)
            vscr = dram.tile([1, N], F32, name="vscr", bufs=1)
            nc.sync.dma_start(out=vscr, in_=vT_sb)
            v_raw = pers.tile([128, KC], F32, name="v_raw")
            nc.sync.dma_start(out=v_raw,
                              in_=vscr[0, :].rearrange("(s p) -> p s", p=128))
            nc.vector.tensor_scalar(out=v_pc, in0=v_raw,
                                    scalar1=1.0 / WV2S, scalar2=float(ATB),
                                    op0=A.mult, op1=A.add)
            # Schraudolph bias: z = 4*log2e*(SCALE*S + v) + 60 + delta
            nc.vector.tensor_scalar(out=w5_pc, in0=v_pc, scalar1=4.0 * LOG2E,
                                    scalar2=60.0 + SCH_DELTA, op0=A.mult, op1=A.add)
            for t in range(QB):
                sl = slice(512 * t, 512 * (t + 1))
                for m in range(CH):
                    ps = pps.tile([128, 512], F32, tag="proj", name=f"psp{t}{m}")
                    for jp in range(CP):
                        nc.tensor.matmul(out=ps, lhsT=m8[:, 2 * jp:2 * jp + 2, 128 * m:128 * (m + 1)],
                                         rhs=xT[:, 2 * jp:2 * jp + 2, sl],
                                         start=(jp == 0), stop=(jp == CP - 1), perf_mode=DRM)
                    if m % 2 == 0:
                        nc.vector.tensor_scalar(out=PT[t][:, m, :], in0=ps,
                                                scalar1=ad32[:, m:m + 1], scalar2=None,
                                                op0=A.mult)
                    else:
                        nc.scalar.activation(out=PT[t][:, m, :], in_=ps, func=AF.Copy,
                                             scale=ad32[:, m:m + 1])

        # ---------- phase 3: attention + output, block-level pipelined ------
        with tc.tile_pool(name="sps", bufs=3, space="PSUM") as sps, \
             tc.tile_pool(name="ops", bufs=1, space="PSUM") as ops, \
             tc.tile_pool(name="seps", bufs=1, space="PSUM") as seps, \
             tc.tile_pool(name="attn", bufs=6) as attnp, \
             tc.tile_pool(name="outp", bufs=3) as outp, \
             tc.tile_pool(name="small", bufs=2) as smallp:

            state = {}

            def attn_loop(bi, qb, q0b, qw, emit_prev):
                """Emit the pair loop for queries [q0b, q0b+qw) of PT block qb,
                interleaving the previous block's output stage (emit_prev
                callbacks) into the early pairs."""
                qoff = q0b - 512 * qb
                ot = [ops.tile([128, 512], F32, tag=f"ot{m}", name=f"ot{m}_{bi}")
                      for m in range(CH)]
                se = seps.tile([128, 512], F32, tag="se", name=f"se_{bi}")
                at_q = {}
                for p in range(KP + 2):
                    if p < KP:
                        at = attnp.tile([128, 2, 512], E4, tag="at", name=f"at_{bi}_{p}")
                        for h in range(2):
                            kc = 2 * p + h
                            sp = sps.tile([128, 512], F32, tag="sp", name=f"sp_{bi}_{kc}")
                            for jp in range(CP):
                                nc.tensor.matmul(out=sp[:, 0:qw], lhsT=xT[:, 2 * jp:2 * jp + 2, 128 * kc:128 * (kc + 1)],
                                                 rhs=PT[qb][:, 2 * jp:2 * jp + 2, qoff:qoff + qw],
                                                 start=(jp == 0), stop=(jp == CP - 1), perf_mode=DRM)
                            sch = SCH_PAIRS.get(p)
                            if sch is None:
                                nc.scalar.activation(out=at[:, h, 0:qw], in_=sp[:, 0:qw],
                                                     func=AF.Exp, scale=SCALE,
                                                     bias=v_pc[:, kc:kc + 1])
                            else:
                                nc.vector.tensor_scalar(out=at[:, h, 0:qw].bitcast(U8), in0=sp[:, 0:qw],
                                                  scalar1=SCALE * 4.0 * LOG2E,
                                                  scalar2=w5_pc[:, kc:kc + 1],
                                                  op0=A.mult, op1=A.add)
                        at_q[p] = at
                    if p >= 2:
                        pp = p - 2
                        atp = at_q.pop(pp)[:, :, 0:qw]
                        if pp in SCH_PAIRS:
                            atp = atp.bitcast(E5)
                        for m in range(CH):
                            nc.tensor.matmul(out=ot[m][:, 0:qw],
                                             lhsT=xtok[:, 2 * pp:2 * pp + 2, 128 * m:128 * (m + 1)],
                                             rhs=atp, start=(pp == 0), stop=(pp == KP - 1),
                                             perf_mode=DRM)
                        nc.tensor.matmul(out=se[:, 0:qw],
                                         lhsT=ones8,
                                         rhs=atp, start=(pp == 0), stop=(pp == KP - 1),
                                         perf_mode=DRM)
                    if emit_prev and 1 <= p <= len(emit_prev):
                        emit_prev[p - 1]()
                state[bi] = (ot, se)

            def out_stage(bi, q0b, qw):
                """Returns closures: [osb drain, rse prep, qw/128 slices]."""
                ot, se = state[bi]
                ns = qw // 128
                rse_pc = smallp.tile([128, 4], F32, tag="rse_pc", name=f"rsepc_{bi}")
                osb = outp.tile([128, CH, 512], E4, tag="osb", name=f"osb_{bi}")

                def drain_osb():
                    for m in range(CH):
                        nc.vector.tensor_scalar(out=osb[:, m, 0:qw], in0=ot[m][:, 0:qw],
                                                scalar1=1.0 / 32.0, scalar2=None,
                                                op0=A.mult)

                def prep_rse():
                    rse = smallp.tile([1, 512], F32, tag="rse", name=f"rse_{bi}")
                    nc.vector.reciprocal(out=rse[:, 0:qw], in_=se[0:1, 0:qw])
                    for s in range(ns):
                        pt = sps.tile([128, 512], F32, tag="sp", name=f"ptr_{bi}_{s}")
                        nc.tensor.transpose(out=pt[:, 0:1], in_=rse[0:1, 128 * s:128 * (s + 1)],
                                            identity=ident1)
                        nc.vector.tensor_copy(out=rse_pc[:, s:s + 1], in_=pt[:, 0:1])

                def slice_s(s):
                    q0 = q0b + 128 * s
                    fp = sps.tile([128, 512], F32, tag="sp", name=f"fp_{q0}")
                    for mp in range(CP):
                        nc.tensor.matmul(out=fp, lhsT=osb[:, 2 * mp:2 * mp + 2, 128 * s:128 * (s + 1)],
                                         rhs=w28[:, 2 * mp:2 * mp + 2, :],
                                         start=(mp == 0), stop=(mp == CP - 1), perf_mode=DRM)
                    res = outp.tile([128, 512], F32, tag="res", name=f"res_{q0}")
                    nc.sync.dma_start(out=res, in_=xres[q0:q0 + 128, :])
                    nc.gpsimd.tensor_add(out=res, in0=res, in1=b2_bc)
                    fo = outp.tile([128, 512], F32, tag="fo", name=f"fo_{q0}")
                    nc.vector.scalar_tensor_tensor(out=fo, in0=fp,
                                                   scalar=rse_pc[:, s:s + 1], in1=res,
                                                   op0=A.mult, op1=A.add)
                    nc.sync.dma_start(out=out[q0:q0 + 128, :], in_=fo)

                return [drain_osb, prep_rse] + [lambda s=s: slice_s(s) for s in range(ns)]

            # 3 full 512-query blocks, then two 256-halves so the first
            # half's output stage hides under the second half's pair loop
            blocks = [(qb, 512 * qb, 512) for qb in range(QB - 1)]
            blocks += [(QB - 1, 512 * (QB - 1), 256), (QB - 1, 512 * (QB - 1) + 256, 256)]
            prev = None
            for bi, (qb, q0b, qw) in enumerate(blocks):
                attn_loop(bi, qb, q0b, qw, prev)
                prev = out_stage(bi, q0b, qw)
            for f in prev:
                f()

    nc.compile()
    return nc


def _swizzle_w(w):
    # [C, C] -> [128, CH, C] bf16 lhsT chunks: [ci_local, ci_chunk, co]
    return np.ascontiguousarray(
        np.asarray(w, np.float32).reshape(CH, 128, C).transpose(1, 0, 2)
    ).astype(np.float16)


def _chunk_pc(v):
    # [C] -> [128, CH]: column j = channels 128j..128j+127
    return np.ascontiguousarray(np.asarray(v, np.float32).reshape(CH, 128).T)


def _in_maps(x, gn_scale, gn_bias, wq, bq, wk, bk, wv, bv, wo, bo):
    e4 = ml_dtypes.float8_e4m3
    gmat = np.zeros((128, 8), np.float32)
    gmat[np.arange(128), np.arange(128) // GS] = 1.0
    wqf = np.asarray(wq, np.float32); wkf = np.asarray(wk, np.float32)
    wvf = np.asarray(wv, np.float32); wof = np.asarray(wo, np.float32)
    M = wqf @ wkf.T               # S = hn M hn^T
    W2 = wvf @ wof                # (attn hn) W2
    g = wkf @ np.asarray(bq, np.float32)          # per-key bias from bq
    h2 = np.asarray(bv, np.float32) @ wof + np.asarray(bo, np.float32)
    wm = np.stack([_swizzle_w(M), _swizzle_w(W2)], axis=1)
    pc = np.concatenate([_chunk_pc(gn_scale), _chunk_pc(gn_bias), _chunk_pc(g), gmat], axis=1)
    common = {
        "wm": np.ascontiguousarray(wm),
        "pc": np.ascontiguousarray(pc.astype(np.float32)),
        "gmat2": np.ascontiguousarray(gmat.T),
        "rows": h2.reshape(1, C).astype(np.float32),
    }
    xf = np.asarray(x, np.float32).reshape(B, N, C)
    in_maps = []
    for core in range(8):
        b, h = core // 2, core % 2
        if h == 0:
            xs = xf[b]
        else:
            xs = np.concatenate([xf[b, NQ:], xf[b, :NQ]], axis=0)
        xt8 = np.ascontiguousarray(xs.T.reshape(CH, 128, N).transpose(1, 0, 2)).astype(e4)
        xk8 = np.ascontiguousarray(xs.reshape(KC, 128, C).transpose(1, 0, 2)).astype(e4)
        in_maps.append({
            **common,
            "xt8": xt8,
            "xk8": xk8,
            "xres": np.ascontiguousarray(xs[:NQ]),
        })
    return in_maps


def kernel(x, gn_scale, gn_bias, wq, bq, wk, bk, wv, bv, wo, bo, _trace=False):
    if "nc" not in _CACHE:
        _CACHE["nc"] = _build()
    nc = _CACHE["nc"]
    in_maps = _in_maps(x, gn_scale, gn_bias, wq, bq, wk, bk, wv, bv, wo, bo)
    last_exc = None
    r = None
    for _attempt in range(4):
        try:
            r = run_bass_kernel_spmd(nc, in_maps, core_ids=list(range(8)), trace=_trace)
            break
        except Exception as e:  # transient NRT/device hiccups: retry
            last_exc = e
            import time as _time
            _time.sleep(3)
    if r is None:
        raise last_exc
    _CACHE["last_result"] = r
    out = np.empty((B, N, C), np.float32)
    for core in range(8):
        b, h = core // 2, core % 2
        out[b, NQ * h:NQ * (h + 1)] = r.results[core]["out"]
    return out.reshape(B, H, W, C)
